# revision 106
# baseline (speedup 1.0000x reference)
"""Multi-head attention (B=4, S=1024, H=1024, 16 heads) on 8 TRN2 NeuronCores.

Sharding: core c = (batch b = c//2, head-group g = c%2). Each core computes
attention for its batch over 8 of the 16 heads (512-wide column slice of the
QKV projections, row slice of Wo). Host sums the two partial output
projections per batch and adds bo.

Per-core dataflow:
  QKV projections as fp8-e4m3 DoubleRow matmuls with 3-term hi/lo error
  compensation (x = xh+xl, W = Wh+Wl host-quantized at pow2 scales sx=16,
  sw=512; descale 2^-13 fused into the psum->SBUF bias pass); product =
  xh*Wh + xl*Wh + xh*Wl.  DoubleRow contracts two 128-chunks per pass at
  0.5 cycles/row -> 0.75x the bf16 matmul cost.
  logitsT[t,s] per head: bf16 Q,K, d=64 contraction, two heads packed in
  the PE via tile_position row groups.
  expT = exp(logitsT/8 + mask*NEG_INF) on ACT -- the single-engine
  bottleneck (64 instructions, one Exp table, nothing else runs on ACT).
  ACT paces the whole head loop, so PE filler work (remaining
  projections, V, AV, transposes) is spread one ~0.6us slice per exp
  chunk with explicit deadlines.
  Startup: weight/x DMAs are packed into few fused transfers (each DMA
  issue costs ~650ns of serial SP time) ordered so the first QT matmuls
  start ~4.3us and head 0 runs half-width (s 0-511) exps from ~9.5us;
  startup descales run on ACT (Identity, shares the Exp table) and DVE
  in parallel, with the startup K psum borrowed from the then-idle lg
  pool so it doesn't rotate behind the Q descales.
  AV reoriented: expT chunk is the STATIONARY operand [128t x 128s], V
  (with a memset 1/64 ones column: softmax denominator) is the moving
  operand [128t x 65] -> out[s-part, d|denom] at 65 rows/pass, half the
  cost of the V-stationary orientation; the denominator lands
  per-partition so normalization is one DVE tensor op.
  attn[s,(h d)] -> attnT[(h d),s]: blocks 0-2 via XBAR DMA transpose
  (14ns/tile, no PE/psum; hi/lo fp8 split on the otherwise-idle GPSIMD),
  block 3 on the PE in the tail (XBAR issue latency would gate it).
  Output projection out[s,n] = attnT^T @ Wo in 3-term fp8-DR, split by
  DR blk-pair: pair0 (blks 0-1) runs as late-loop PE filler into bf16
  partials (nh0 pre-scaled for a DVE stt merge, nh1 raw for a PE
  identity-matmul re-inject + ACT copy); pair1 + merges + per-st DMAs
  form the tail. Logits are emitted before each slot's fillers (the
  32-deep in-order PE window would otherwise delay them ~0.8us behind a
  32-matmul av unit and starve ACT).
"""
import sys

sys.path.insert(0, "/opt/trn_rl_repo")

import ml_dtypes
import numpy as np

import concourse.bass as bass
import concourse.mybir as mybir
import concourse.tile as tile
from concourse import bacc
from concourse.bass_utils import run_bass_kernel_spmd

F32 = mybir.dt.float32
BF16 = mybir.dt.bfloat16
E4 = mybir.dt.float8e4
DR = mybir.MatmulPerfMode.DoubleRow
Exp = mybir.ActivationFunctionType.Exp
Ident = mybir.ActivationFunctionType.Identity
MULT = mybir.AluOpType.mult
ADD = mybir.AluOpType.add

B, S, H = 4, 1024, 1024
NH, HD = 16, 64
HPG = 8            # heads per group (per core)
GW = HPG * HD      # 512
NEG_INF = -2.0 ** 32
NCORES = 8
HC = H // 128      # 8 contraction chunks over hidden
TC = S // 128      # 8 chunks over key positions t
SX = 16.0          # x fp8 scale
SW = 512.0         # W fp8 scale
DESCALE = 1.0 / (SX * SW)   # 2^-13


def _build(nrep=1):
    nc = bacc.Bacc("TRN2", target_bir_lowering=False, debug=False)

    # weight layouts are blk-major so per-blk DMA slices are contiguous
    # >=512B runs (sub-512B DMA elements pay 2x latency). x and the qk
    # weights are fused hi|lo / q|k so the startup needs few DMA issues
    # (each costs 650ns of serial SP issue time).
    # x packed as hi|lo-FUSED 256-column pieces (outermost piece dim =>
    # each piece is one contiguous 4KB/partition DMA): all three DR terms
    # of a projection sub-block unlock together as its piece lands, so the
    # PE runs continuously through the startup instead of idling for the
    # lo plane. w0 is chunk-major so one DMA carries wq+wk hi+lo for a
    # chunk range.
    xhl = nc.dram_tensor("xhl", [128, 4, 2, HC, 256], E4,
                         kind="ExternalInput")
    w0 = nc.dram_tensor("w0", [128, HC, 4, 128], E4, kind="ExternalInput")
    wqk = nc.dram_tensor("wqk", [128, 3, 4, HC, 128], E4, kind="ExternalInput")
    wv = nc.dram_tensor("wv", [128, 2, 2, HC, 256], E4, kind="ExternalInput")
    wo = nc.dram_tensor("wo", [128, 2, 4, S], E4, kind="ExternalInput")
    # consts packs [bq | bk | mask] as [128, 4+4+8] (one DMA instead of 3)
    consts = nc.dram_tensor("consts", [128, 16], F32, kind="ExternalInput")
    bv1 = nc.dram_tensor("bv1", [1, GW], F32, kind="ExternalInput")
    ident = nc.dram_tensor("ident", [128, 128], BF16, kind="ExternalInput")
    out = nc.dram_tensor("out", [S, H], BF16, kind="ExternalOutput")

    with tile.TileContext(nc, pool_alloc_mode="stack") as tc:
      for _rep in range(nrep):
        misc_cm = tc.tile_pool(name="misc", bufs=1); misc = misc_cm.__enter__()
        x_cm = tc.tile_pool(name="p_x", bufs=1); p_x = x_cm.__enter__()
        w_cm = tc.tile_pool(name="p_w", bufs=1); p_w = w_cm.__enter__()
        qkt_cm = tc.tile_pool(name="p_qkt", bufs=1); p_qkt = qkt_cm.__enter__()
        v_cm = tc.tile_pool(name="p_v", bufs=1); p_v = v_cm.__enter__()
        exp_cm = tc.tile_pool(name="p_exp", bufs=3); p_exp = exp_cm.__enter__()
        attn_cm = tc.tile_pool(name="p_attn", bufs=1); p_attn = attn_cm.__enter__()
        r4_cm = tc.tile_pool(name="p_r4", bufs=2); p_r4 = r4_cm.__enter__()
        at_cm = tc.tile_pool(name="p_at", bufs=1); p_at = at_cm.__enter__()
        o_cm = tc.tile_pool(name="p_o", bufs=8); p_o = o_cm.__enter__()
        # PSUM: proj(2 banks) + po(2) + lg(4) = 8. lg releases after the
        # head loop; the output-projection pool takes its 4 banks. po stays
        # for the tail transposes (borrowed slots).
        prps_cm = tc.tile_pool(name="ps_proj", bufs=2, space="PSUM")
        ps_proj = prps_cm.__enter__()
        pops_cm = tc.tile_pool(name="ps_po", bufs=2, space="PSUM")
        ps_po = pops_cm.__enter__()
        lgps_cm = tc.tile_pool(name="ps_lg", bufs=2, space="PSUM")
        ps_lg = lgps_cm.__enter__()

        xhl_sb = p_x.tile([128, 4, 2, HC, 256], E4, tag="xhl")
        w0_sb = p_w.tile([128, HC, 4, 128], E4, tag="w0")
        wqk_sb = p_w.tile([128, 3, 4, HC, 128], E4, tag="wqk")
        wv_sb = p_w.tile([128, 2, 2, HC, 256], E4, tag="wv")
        wo_sb = p_w.tile([128, 2, 4, S], E4, tag="wo")
        woh_sb = wo_sb[:, 0]
        wol_sb = wo_sb[:, 1]
        c_sb = misc.tile([128, 16], F32, tag="consts")
        bq_sb = c_sb[:, 0:4]
        bk_sb = c_sb[:, 4:8]
        maskb = misc.tile([128, TC], F32, tag="maskb")
        bv_bc = misc.tile([128, GW], F32, tag="bv")
        id_sb = misc.tile([128, 128], BF16, tag="ident")

        def w_ap(mb, j):
            """Weight blk mb, slot j (0=wqh 1=wql 2=wkh 3=wkl) as
            [128, HC, 128]."""
            if mb == 0:
                return w0_sb[:, :, j, :]
            return wqk_sb[:, mb - 1, j]

        def dma(dst, src):
            nc.sync.dma_start(out=dst, in_=src)

        # Preload the Exp activation table off the critical path: the first
        # real exp would otherwise pay the 1.28us table load at ~10us.
        dummy = misc.tile([128, 1], F32, tag="dummy")
        nc.vector.memset(dummy, 0.0)
        nc.scalar.activation(out=dummy, in_=dummy, func=Exp)

        # Startup DMA queue: the first piece (wqh chunks 0-3) plus the
        # matching xh quarter let the very first QT matmuls start ~4.3us;
        # kernel end time tracks PE-start + PE-busy, so this is critical.
        dma(w0_sb[:, 0:2], w0[:, 0:2])
        dma(xhl_sb[:, 0, :, 0:2, :], xhl[:, 0, :, 0:2, :])
        dma(w0_sb[:, 2:8], w0[:, 2:8])
        dma(xhl_sb[:, 0, :, 2:8, :], xhl[:, 0, :, 2:8, :])
        dma(c_sb, consts[:, :])
        nc.vector.tensor_scalar_mul(maskb, c_sb[:, 8:16], NEG_INF)
        dma(xhl_sb[:, 1], xhl[:, 1])
        dma(xhl_sb[:, 2], xhl[:, 2])
        dma(xhl_sb[:, 3], xhl[:, 3])
        # blk1 weights: QK-mb1 filler halves start at h0c4
        dma(wqk_sb[:, 0], wqk[:, 0])
        # wv first half (heads 0-3): V fillers start at h1c4
        dma(wv_sb[:, 0], wv[:, 0])
        dma(bv_bc, bv1[0:1, :].to_broadcast((128, GW)))
        dma(id_sb, ident[:, :])
        QT = p_qkt.tile([128, 4, S], BF16, tag="QT")
        KT = p_qkt.tile([128, 4, S], BF16, tag="KT")
        V_sb = p_v.tile([128, TC, HPG, HD + 1], BF16, tag="V")
        # softmax-denominator ones column (1/64 scale), built on DVE instead
        # of a 64-descriptor-per-partition DMA (3.6us of DMA queue time)
        nc.vector.memset(V_sb[:, :, :, HD:HD + 1], 1.0 / 64.0)
        dma(wqk_sb[:, 1], wqk[:, 1])
        dma(wqk_sb[:, 2], wqk[:, 2])
        # wv second half (heads 4-7): V-b fillers start at h4c3
        dma(wv_sb[:, 1], wv[:, 1])
        dma(wo_sb, wo[:, :])
        attn_sb = p_attn.tile([128, TC, GW], BF16, tag="attn")
        # attnT: bf16 XBAR-transpose staging + hi/lo fp8 split. Blocks 0-2
        # transpose via DMA (XBAR, 14ns/tile, idle DMA engines mid-loop);
        # block 3 transposes on the PE in the tail (DMA issue latency
        # ~2.9us would gate the tail otherwise).
        attnT_bf = p_at.tile([128, 4, S], BF16, tag="attnTbf")
        attnT_h = p_at.tile([128, 4, S], E4, tag="attnTh")
        attnT_l = p_at.tile([128, 4, S], E4, tag="attnTl")
        # output-projection partial sums (pair0 = attnT blks 0-1, done
        # mid-loop; merged with the tail pair1 psum): [st][nh] -> [128,512]
        part_sb = p_at.tile([128, TC, 2, 512], BF16, tag="part")

        # ---- compute emitters ----
        pq_state = {}

        def proj_qk_half(dst, b_sb, jh, mb, nh, nb, split=False,
                         desc_act=False, use_lg=False):
            """Half (256 s-cols) of a 3-term fp8-DR projection block.
            nb=0 allocates the [128,512] psum tile; nb=1 emits the fused
            descale+bias DVE pass over the full 512. With split=True
            (startup halves) each nb descales its own 256 as soon as its
            group stops -- nb0 on ACT (Copy shares the Exp table), nb1 on
            DVE -- and the xh terms are emitted before the xl terms so
            they flow while the xl half of x is still in flight."""
            wh_t, wl_t = w_ap(mb, jh), w_ap(mb, jh + 1)
            key = (id(dst), mb, nh)
            if split:
                # own psum tile per nb: sharing one tile would serialize
                # nb1's accumulation group behind the engine reading nb0's
                # descale out of the same bank. The K halves of the very
                # first chain borrow the (startup-idle) lg pool so they
                # don't rotate behind the Q descales in the proj pool.
                if use_lg:
                    ps = ps_lg.tile([128, 1024], F32, tag="lg",
                                    name=f"pq{jh}{mb}_{nh}_{nb}")
                    ps = ps[:, 0:512]
                else:
                    ps = ps_proj.tile([128, 512], F32, tag="pproj",
                                      name=f"pq{jh}{mb}_{nh}_{nb}")
            else:
                if nb == 0:
                    pq_state[key] = ps_proj.tile(
                        [128, 512], F32, tag="pproj", name=f"pq{jh}{mb}_{nh}")
                ps = pq_state[key]
            n0 = nh * 512
            pp = nh * 2 + nb   # x piece index (256 s-cols, hi|lo fused)
            os_ = slice(0, 256) if split else slice(nb * 256, (nb + 1) * 256)
            terms = ((wh_t, 0), (wl_t, 0), (wh_t, 1))
            if split:
                order = [(cp, t) for t in terms for cp in range(4)]
            else:
                order = [(cp, t) for cp in range(4) for t in terms]
            for i, (cp, (wt, hl)) in enumerate(order):
                cs = slice(2 * cp, 2 * cp + 2)
                nc.tensor.matmul(
                    ps[:, os_], wt[:, cs, :], xhl_sb[:, pp, hl, cs, :],
                    start=(i == 0), stop=(i == len(order) - 1), perf_mode=DR,
                    skip_group_check=True)
            if split:
                od = dst[:, mb, n0 + nb * 256:n0 + (nb + 1) * 256]
                if nb == 0 and desc_act:
                    # ACT-hosted descale: only before the first exp, while
                    # ACT is idle (later it would queue behind exps)
                    nc.scalar.activation(out=od, in_=ps[:, os_], func=Ident,
                                         bias=b_sb[:, mb:mb + 1],
                                         scale=DESCALE)
                else:
                    nc.vector.tensor_scalar(
                        od, ps[:, os_], DESCALE, b_sb[:, mb:mb + 1],
                        MULT, ADD)
            elif nb == 1:
                nc.vector.tensor_scalar(
                    dst[:, mb, n0:n0 + 512], ps, DESCALE,
                    b_sb[:, mb:mb + 1], MULT, ADD)
                del pq_state[key]

        def proj_v_half(tb, nb):
            """V rows 128*tb for head-group nb (4 heads, 256 wv-cols);
            self-contained: 12 DR matmuls + fused descale+bias pass."""
            ps = ps_proj.tile([128, 512], F32, tag="pproj",
                              name=f"pv{tb}_{nb}")
            pp, to = tb // 2, (tb % 2) * 128
            ts = slice(to, to + 128)
            first = True
            for cp in range(4):
                cs = slice(2 * cp, 2 * cp + 2)
                for wl_, xl_ in ((0, 0), (1, 0), (0, 1)):
                    last = (cp == 3 and xl_ == 1)
                    nc.tensor.matmul(
                        ps[:, 0:256], xhl_sb[:, pp, xl_, cs, ts],
                        wv_sb[:, nb, wl_, cs, :],
                        start=first, stop=last, perf_mode=DR,
                        skip_group_check=True)
                    first = False
            nc.vector.scalar_tensor_tensor(
                V_sb[:, tb, 4 * nb:4 * nb + 4, 0:HD],
                ps[:, 0:256].rearrange("p (h d) -> p h d", h=4), DESCALE,
                bv_bc[:, nb * 256:(nb + 1) * 256].rearrange(
                    "p (h d) -> p h d", h=4), MULT, ADD)

        def logits_exp(h, eT, tcn):
            mb, off = h // 2, (h % 2) * 64
            lg = ps_lg.tile([128, 1024], F32, tag="lg")
            for sh in range(2):
                nc.tensor.matmul(
                    lg[:, sh * 512:(sh + 1) * 512],
                    KT[off:off + 64, mb, tcn * 128:(tcn + 1) * 128],
                    QT[off:off + 64, mb, sh * 512:(sh + 1) * 512],
                    start=True, stop=True, tile_position=(off, 0))
            nc.scalar.activation(
                out=eT[:, tcn, :], in_=lg, func=Exp,
                bias=maskb[:, tcn:tcn + 1], scale=0.125)

        expT = {}
        av_state = {}

        def av_part(h, sbg, tcs):
            """Partial AV accumulation for head h, s-blocks 4*sbg.., over
            the t-chunks in `tcs` (split emission so av(7)'s last chunk is
            the only tail PE work)."""
            key = (h, sbg)
            if key not in av_state:
                av_state[key] = ps_po.tile(
                    [128, 4, HD + 1], F32, tag="po", name=f"po{h}_{sbg}")
            po = av_state[key]
            eT = expT[h]
            for i in range(4):
                sb = sbg * 4 + i
                for tcn in tcs:
                    nc.tensor.matmul(
                        po[:, i, :],
                        eT[:, tcn, sb * 128:(sb + 1) * 128],
                        V_sb[:, tcn, h, :],
                        start=(tcn == 0), stop=(tcn == TC - 1))

        def av_fin(h, sbg):
            po = av_state.pop((h, sbg))
            r4 = p_r4.tile([128, 4, 1], F32, tag="r4")
            nc.vector.reciprocal(r4, po[:, :, HD:HD + 1])
            nc.vector.tensor_mul(
                attn_sb[:, sbg * 4:(sbg + 1) * 4, h * HD:(h + 1) * HD],
                po[:, :, 0:HD], r4[:, :, 0:1].to_broadcast((128, 4, HD)))

        def av(h, sbg):
            av_part(h, sbg, range(TC))
            av_fin(h, sbg)

        def transpose_blk(blk):
            """attn s-block columns of gw-block blk -> attnT hi/lo fp8 rows
            (attn carries a x64 pow2 scale from the 1/64 ones column so the
            values sit in e4m3's normal range; descale folds into the
            output-copy pass). XBAR DMA transpose (no PE, no psum); hi
            split on DVE, lo residual on the otherwise-idle GPSIMD."""
            for sb in range(TC):
                ss = slice(sb * 128, (sb + 1) * 128)
                nc.sync.dma_start_transpose(
                    attnT_bf[:, blk, ss],
                    attn_sb[:, sb, blk * 128:(blk + 1) * 128])
                # both converts on GPSIMD: the ~2.9us XBAR round-trip sits
                # at the head of whichever in-order engine queue hosts the
                # first convert; GPSIMD's queue has nothing time-critical,
                # while a DVE-hosted wait would stall the descale pipeline
                # (psum recycling) behind it
                nc.gpsimd.tensor_copy(attnT_h[:, blk, ss],
                                      attnT_bf[:, blk, ss])
                nc.gpsimd.tensor_sub(attnT_l[:, blk, ss],
                                     attnT_bf[:, blk, ss],
                                     attnT_h[:, blk, ss])

        ODESC = 1.0 / (64.0 * SW)   # attn x64 and wo x512 scales

        def oproj_pair(st, nh, pair, merge_to=None, merge_eng="v", pool=None):
            """6 fp8-DR matmuls of output-projection pair `pair` (attnT
            blk-pair) for s-chunk st, columns nh*512.. .
            pair0 (mid-loop filler): psum -> bf16 partial on DVE. nh0
            stores the partial pre-scaled by ODESC (tail merge is a DVE
            scalar_tensor_tensor); nh1 stores it raw (the tail re-injects
            it into psum via an identity matmul -- GPSIMD can't read PSUM,
            so ACT does that lane's plain scaled copy instead).
            pair1 (tail): per merge_eng "v": DVE stt psum*ODESC+partial;
            "inject": identity-matmul the raw partial into the psum group
            first, then a plain ACT copy*ODESC."""
            sts = slice(st * 128, (st + 1) * 128)
            op = (pool or ps_proj).tile([128, 512], F32, tag="pproj"
                                        if pool is None else "op",
                                        name=f"op{st}_{nh}_{pair}")
            bs = slice(2 * pair, 2 * pair + 2)
            first = True
            if merge_to is not None and merge_eng == "inject":
                nc.tensor.matmul(op, id_sb, part_sb[:, st, nh],
                                 start=True, stop=False,
                                 skip_group_check=True)
                first = False
            for nb in range(2):
                ws = slice(nh * 512 + nb * 256, nh * 512 + (nb + 1) * 256)
                os_ = slice(nb * 256, (nb + 1) * 256)
                for at_t, wo_t in ((attnT_h, woh_sb), (attnT_l, woh_sb),
                                   (attnT_h, wol_sb)):
                    last = (nb == 1 and wo_t is wol_sb)
                    nc.tensor.matmul(
                        op[:, os_], at_t[:, bs, sts], wo_t[:, bs, ws],
                        start=first, stop=last, perf_mode=DR,
                        skip_group_check=True)
                    first = False
            if merge_to is None:
                # partial copy on DVE (NOT Pool: the in-loop proj-psum
                # recycling waits on this copy, Pool's in-order queue is
                # full of non-urgent tp converts, and GPSIMD can't read
                # PSUM on real hardware anyway)
                if nh == 0:
                    nc.vector.tensor_scalar_mul(part_sb[:, st, nh], op,
                                                ODESC)
                else:
                    nc.vector.tensor_copy(part_sb[:, st, nh], op)
            elif merge_eng == "inject":
                nc.scalar.mul(merge_to, op, ODESC)
            else:
                nc.vector.scalar_tensor_tensor(
                    merge_to, op, ODESC, part_sb[:, st, nh], MULT, ADD)

        # ---------------- emission ----------------
        # Startup: QT-nh0 + KT-nb0 (x half0 only) gate half-width exps
        # (s 0-511) for head 0's first four t-chunks, ~4us before a
        # full-width first exp could run (that needs x half1 for QT-nh1).
        # The sh1 half-exps follow once QT-nh1 is projected; tc4-7 run
        # full width. Half-exps pay one extra ACT access-latency each
        # (~0.2us total) for the earlier start.
        def logits_exp_half(h, eT, tcn, sh):
            mb, off = h // 2, (h % 2) * 64
            lg = ps_lg.tile([128, 1024], F32, tag="lg")
            nc.tensor.matmul(
                lg[:, 0:512],
                KT[off:off + 64, mb, tcn * 128:(tcn + 1) * 128],
                QT[off:off + 64, mb, sh * 512:(sh + 1) * 512],
                start=True, stop=True, tile_position=(off, 0))
            nc.scalar.activation(
                out=eT[:, tcn, sh * 512:(sh + 1) * 512], in_=lg[:, 0:512],
                func=Exp, bias=maskb[:, tcn:tcn + 1], scale=0.125)

        expT[0] = p_exp.tile([128, TC, S], BF16, tag="expT", name="expT0")
        proj_qk_half(QT, bq_sb, 0, 0, 0, 0, split=True, desc_act=True)
        proj_qk_half(QT, bq_sb, 0, 0, 0, 1, split=True)
        proj_qk_half(KT, bk_sb, 2, 0, 0, 0, split=True, desc_act=True,
                     use_lg=True)
        logits_exp_half(0, expT[0], 0, 0)
        logits_exp_half(0, expT[0], 1, 0)
        proj_qk_half(KT, bk_sb, 2, 0, 0, 1, split=True, use_lg=True)
        logits_exp_half(0, expT[0], 2, 0)
        logits_exp_half(0, expT[0], 3, 0)
        proj_qk_half(QT, bq_sb, 0, 0, 1, 0)
        proj_qk_half(QT, bq_sb, 0, 0, 1, 1)
        logits_exp_half(0, expT[0], 0, 1)
        logits_exp_half(0, expT[0], 1, 1)
        proj_qk_half(KT, bk_sb, 2, 0, 1, 0)
        logits_exp_half(0, expT[0], 2, 1)
        logits_exp_half(0, expT[0], 3, 1)
        proj_qk_half(KT, bk_sb, 2, 0, 1, 1)

        def F_qk(dst_b, mb, nh, nb):
            dst, b_, jh = (QT, bq_sb, 0) if dst_b == "q" else (KT, bk_sb, 2)
            return lambda: proj_qk_half(dst, b_, jh, mb, nh, nb)

        def F_v(tb, nb):
            return lambda: proj_v_half(tb, nb)

        def F_av(h, g):
            return lambda: av(h, g)

        def F_avp(h, g, tcs):
            return lambda: av_part(h, g, tcs)

        def F_tp(blk):
            return lambda: transpose_blk(blk)

        def F_op0(st, nh):
            return lambda: oproj_pair(st, nh, 0)

        # filler schedule: [h][tcn] -> list of emitters. Budget ~0.6us of
        # PE filler per 1.04us exp chunk; no fillers on the first chunks
        # (their inputs aren't DMA'd yet and PE head-of-line blocking
        # would starve ACT). Deadlines: QT/KT-mb(k) before head 2k's
        # logits, V head-group a (0-3) before av(0) at h2c5, group b
        # before av(4) at h6c4, av(h) done before head h+3 starts (expT
        # pool bufs=3), transpose blk b right after av(2b+1) (XBAR DMA +
        # DVE/GPSIMD only -- no PE cost), output-projection pair0 groups
        # (attnT blks 0-1) as late-loop PE fillers once tp0/tp1 are done,
        # av(7) t-chunks 0-6 in-loop so only its last chunk trails the
        # final exp.
        FILL = {
            (0, 4): [F_qk("q", 1, 0, 0)],
            (0, 5): [F_qk("q", 1, 0, 1)],
            (0, 6): [F_qk("q", 1, 1, 0)],
            (0, 7): [F_qk("q", 1, 1, 1)],
            (1, 0): [F_qk("k", 1, 0, 0)],
            (1, 1): [F_qk("k", 1, 0, 1)],
            (1, 2): [F_qk("k", 1, 1, 0)],
            (1, 3): [F_qk("k", 1, 1, 1)],
            (1, 4): [F_v(0, 0)], (1, 5): [F_v(1, 0)],
            (1, 6): [F_v(2, 0)], (1, 7): [F_v(3, 0)],
            (2, 0): [F_v(4, 0)], (2, 1): [F_v(5, 0)],
            (2, 2): [F_v(6, 0)], (2, 3): [F_v(7, 0)],
            (2, 4): [F_qk("q", 2, 0, 0)],
            (2, 5): [F_av(0, 0)],
            (2, 6): [F_qk("q", 2, 0, 1)],
            (2, 7): [F_av(0, 1)],
            (3, 0): [F_qk("q", 2, 1, 0)],
            (3, 1): [F_av(1, 0)],
            (3, 2): [F_qk("q", 2, 1, 1)],
            (3, 3): [F_qk("k", 2, 0, 0)],
            (3, 4): [F_qk("k", 2, 0, 1)],
            (3, 5): [F_av(1, 1)],
            (3, 6): [F_qk("k", 2, 1, 0), F_tp(0)],
            (3, 7): [F_qk("k", 2, 1, 1)],
            (4, 0): [F_qk("q", 3, 0, 0)],
            (4, 1): [F_av(2, 0)],
            (4, 2): [F_qk("q", 3, 0, 1)],
            (4, 3): [F_v(0, 1)],
            (4, 4): [F_qk("q", 3, 1, 0)],
            (4, 5): [F_av(2, 1)],
            (4, 6): [F_qk("q", 3, 1, 1)],
            (4, 7): [F_v(1, 1)],
            (5, 0): [F_qk("k", 3, 0, 0)],
            (5, 1): [F_av(3, 0)],
            (5, 2): [F_qk("k", 3, 0, 1)],
            (5, 3): [F_qk("k", 3, 1, 0)],
            (5, 4): [F_qk("k", 3, 1, 1)],
            (5, 5): [F_av(3, 1)],
            (5, 6): [F_v(2, 1), F_tp(1)],
            (5, 7): [F_v(3, 1)],
            (6, 0): [F_v(4, 1)],
            (6, 1): [F_v(5, 1)],
            (6, 2): [F_v(6, 1)],
            (6, 3): [F_v(7, 1)],
            (6, 4): [F_av(4, 0)],
            (6, 5): [F_op0(0, 0)],
            (6, 6): [F_av(4, 1)],
            (6, 7): [F_op0(0, 1)],
            (7, 0): [F_av(5, 0)],
            (7, 1): [F_av(5, 1)],
            (7, 2): [F_av(6, 0), F_tp(2)],
            (7, 3): [F_av(6, 1)],
            (7, 4): [F_op0(1, 0), F_op0(1, 1)],
            (7, 5): [F_op0(2, 0), F_op0(2, 1)],
            (7, 6): [F_op0(3, 0), F_op0(3, 1)],
            (7, 7): [F_op0(4, 0), F_op0(4, 1)],
        }

        # logits first within each slot: they enter the PE window ahead of
        # the slot's fillers (the PE drains ready work in dispatch order,
        # so a logits pair emitted after a 32-matmul av unit would wait
        # ~0.86us behind it and starve ACT)
        for h in range(HPG):
            if h > 0:
                expT[h] = p_exp.tile([128, TC, S], BF16, tag="expT",
                                     name=f"expT{h}")
            for tcn in range(TC):
                if h == 0 and tcn < 4:
                    continue  # emitted above as half-width startup exps
                logits_exp(h, expT[h], tcn)
                for f in FILL.get((h, tcn), ()):
                    f()

        lgps_cm.__exit__(None, None, None)
        opps_cm = tc.tile_pool(name="ps_op", bufs=4, space="PSUM")
        ps_op = opps_cm.__enter__()

        def transpose_one3(sb):
            # tail blk3 transposes on the PE (XBAR DMA issue latency
            # ~2.9us/8-issue-serial would gate the tail): psum slots
            # alternate between the AV pool (free after av7) and the proj
            # pool for a 4-wide pipeline; hi copy on ACT (idle after the
            # last exp; Copy shares the Exp table, no reload), lo subtract
            # on DVE.
            if sb % 2:
                bt4 = ps_po.tile([128, 4, HD + 1], F32, tag="po",
                                 name=f"pt3_{sb}")
                pt = bt4[:, 0, :].bitcast(BF16)[:, 0:128]
            else:
                bt = ps_proj.tile([128, 512], F32, tag="pproj",
                                  name=f"pt3_{sb}")
                pt = bt[:, 0:128].bitcast(BF16)[:, 0:128]
            nc.tensor.matmul(
                pt, attn_sb[:, sb, 384:512], id_sb,
                start=True, stop=True, is_transpose=True)
            ss = slice(sb * 128, (sb + 1) * 128)
            nc.scalar.copy(attnT_h[:, 3, ss], pt)
            nc.vector.tensor_sub(attnT_l[:, 3, ss], pt, attnT_h[:, 3, ss])

        # av7: both sbg matmul groups, then recips back-to-back and muls
        # after (one DVE serial chain, no interleaved stalls)
        av(7, 0)
        av(7, 1)
        # full-group sts first: their transposes and output DMAs drain
        # while the short pair1+merge sts still compute
        ST_ORDER = [0, 1, 2, 3, 4, 5, 6, 7]
        for sb in ST_ORDER:
            transpose_one3(sb)

        # tail: sts whose pair0 ran as loop filler finish with pair1 +
        # merge (DVE nh0 / GPSIMD nh1); the rest run full 12-matmul
        # groups with plain copies (DVE nh0 / ACT nh1), all in the 4-bank
        # op pool.
        PAIR0_DONE = 5  # sts 0..PAIR0_DONE-1 had pair0 emitted in-loop
        for st in ST_ORDER:
            sts = slice(st * 128, (st + 1) * 128)
            o_sb = p_o.tile([128, 1024], BF16, tag="o")
            if st < PAIR0_DONE:
                oproj_pair(st, 0, 1, merge_to=o_sb[:, 0:512],
                           merge_eng="v", pool=ps_op)
                oproj_pair(st, 1, 1, merge_to=o_sb[:, 512:1024],
                           merge_eng="inject", pool=ps_op)
            else:
                for nh in range(2):
                    # nh1 groups borrow the proj pool (free once the tail
                    # transposes drain) to widen the psum pipeline
                    pool_f = ps_op if nh == 0 else ps_proj
                    op = pool_f.tile([128, 512], F32,
                                     tag="op" if nh == 0 else "pproj",
                                     name=f"opf{st}_{nh}")
                    first = True
                    for nb in range(2):
                        ws = slice(nh * 512 + nb * 256,
                                   nh * 512 + (nb + 1) * 256)
                        os_ = slice(nb * 256, (nb + 1) * 256)
                        for p_ in range(2):
                            for at_t, wo_t in ((attnT_h, woh_sb),
                                               (attnT_l, woh_sb),
                                               (attnT_h, wol_sb)):
                                last = (nb == 1 and p_ == 1
                                        and wo_t is wol_sb)
                                nc.tensor.matmul(
                                    op[:, os_], at_t[:, 2 * p_:2 * p_ + 2, sts],
                                    wo_t[:, 2 * p_:2 * p_ + 2, ws],
                                    start=first, stop=last, perf_mode=DR,
                                    skip_group_check=True)
                                first = False
                    cs2 = slice(nh * 512, (nh + 1) * 512)
                    if nh == 0:
                        nc.vector.tensor_scalar_mul(o_sb[:, cs2], op, ODESC)
                    else:
                        nc.scalar.mul(o_sb[:, cs2], op, ODESC)
            if st != ST_ORDER[-1]:
                dma(out[sts, :], o_sb)
            else:
                # last chunk: per-half DMAs so the final transfer (and the
                # post-DMA fixed latency chain) is half-sized
                dma(out[sts, 0:512], o_sb[:, 0:512])
                dma(out[sts, 512:1024], o_sb[:, 512:1024])

        for cm in (opps_cm, pops_cm, prps_cm, o_cm, at_cm, r4_cm, attn_cm,
                   exp_cm, v_cm, qkt_cm, w_cm, x_cm, misc_cm):
            cm.__exit__(None, None, None)

    nc.compile()
    return nc


_NC = {}


def _get_nc(nrep=1):
    if nrep not in _NC:
        _NC[nrep] = _build(nrep)
    return _NC[nrep]


E4NP = ml_dtypes.float8_e4m3


def _q8(a):
    """fp8 hi/lo split: a ~= hi + lo (both e4m3)."""
    hi = a.astype(E4NP)
    lo = (a - hi.astype(np.float32)).astype(E4NP)
    return hi, lo


def _chunk128(a):
    """[1024, M] -> [128, 8, M] partition-major chunking of the rows."""
    m = a.shape[1]
    return np.ascontiguousarray(a.reshape(HC, 128, m).transpose(1, 0, 2))


def kernel(x, mask, Wq, bq, Wk, bk, Wv, bv, Wo, bo, _trace=False):
    x = np.asarray(x, dtype=np.float32)
    mask = np.asarray(mask, dtype=np.float32)
    Wq, Wk, Wv, Wo = (np.asarray(w, dtype=np.float32) for w in (Wq, Wk, Wv, Wo))
    bq, bk, bv, bo = (np.asarray(b_, dtype=np.float32) for b_ in (bq, bk, bv, bo))

    nc = _get_nc()
    # The on-device ones column is memset to 1/64: the softmax denominator
    # comes out pre-scaled so the normalized attn carries a x64 factor,
    # putting it in e4m3's normal range for the fp8 output projection
    # (descaled by ODESC at the end).
    ident = np.eye(128, dtype=ml_dtypes.bfloat16)
    in_maps = []
    for c in range(NCORES):
        b, g = c // 2, c % 2
        sl = slice(g * GW, (g + 1) * GW)
        xh_, xl_ = _q8(np.ascontiguousarray(x[b].T) * SX)
        wq_h, wq_l = _q8(Wq[:, sl] * SW)
        wk_h, wk_l = _q8(Wk[:, sl] * SW)
        wv_h, wv_l = _q8(Wv[:, sl] * SW)
        wo_h, wo_l = _q8(np.ascontiguousarray(
            Wo[sl, :].reshape(4, 128, S).transpose(1, 0, 2)) * SW)

        def wblk(a, nblk):
            # [1024, 512] -> [128, nblk, HC, 512//nblk] (partition-major
            # rows, blk-major cols so per-blk DMA slices are contiguous)
            cc = a.reshape(HC, 128, nblk, GW // nblk)
            return np.ascontiguousarray(cc.transpose(1, 2, 0, 3))

        qh, ql = wblk(wq_h, 4), wblk(wq_l, 4)
        kh, kl = wblk(wk_h, 4), wblk(wk_l, 4)
        vh, vl = wblk(wv_h, 2), wblk(wv_l, 2)
        # w0: blk0 of [wqh|wql|wkh|wkl] chunk-major; wqk: blks 1-3 stacked
        w0 = np.stack([a[:, 0] for a in (qh, ql, kh, kl)], axis=2)
        wqk = np.stack(
            [np.stack([a[:, blk] for a in (qh, ql, kh, kl)], axis=1)
             for blk in range(1, 4)], axis=1)
        wv_f = np.stack(
            [np.stack([vh[:, nb], vl[:, nb]], axis=1) for nb in range(2)],
            axis=1)
        # x as 4 hi|lo-fused 256-column pieces: [128, 4, 2, HC, 256]
        xhl = np.stack(
            [_chunk128(xh_).reshape(128, HC, 4, 256).transpose(0, 2, 1, 3),
             _chunk128(xl_).reshape(128, HC, 4, 256).transpose(0, 2, 1, 3)],
            axis=2)

        in_maps.append({
            "xhl": np.ascontiguousarray(xhl),
            "w0": np.ascontiguousarray(w0),
            "wqk": np.ascontiguousarray(wqk),
            "wv": np.ascontiguousarray(wv_f),
            "wo": np.ascontiguousarray(np.stack([wo_h, wo_l], axis=1)),
            "consts": np.ascontiguousarray(np.concatenate([
                bq[sl].reshape(4, 128).T, bk[sl].reshape(4, 128).T,
                mask[b, 0, 0, :].reshape(8, 128).T], axis=1)),
            "bv1": np.ascontiguousarray(bv[sl]).reshape(1, GW),
            "ident": ident,
        })
    # First execution after NEFF load can race engine table initialization.
    # Warm up, then run.
    run_bass_kernel_spmd(nc, in_maps, core_ids=list(range(NCORES)))
    res = run_bass_kernel_spmd(
        nc, in_maps, core_ids=list(range(NCORES)), trace=_trace)
    kernel.last_results = res
    parts = [res.results[c]["out"].astype(np.float32) for c in range(NCORES)]
    return np.stack(
        [parts[2 * b] + parts[2 * b + 1] + bo for b in range(B)]
    ).astype(np.float32)


# revision 108
# speedup vs baseline: 1.0018x; 1.0018x over previous
"""Multi-head attention (B=4, S=1024, H=1024, 16 heads) on 8 TRN2 NeuronCores.

Sharding: core c = (batch b = c//2, head-group g = c%2). Each core computes
attention for its batch over 8 of the 16 heads (512-wide column slice of the
QKV projections, row slice of Wo). Host sums the two partial output
projections per batch and adds bo.

Per-core dataflow:
  QKV projections as fp8-e4m3 DoubleRow matmuls with 3-term hi/lo error
  compensation (x = xh+xl, W = Wh+Wl host-quantized at pow2 scales sx=16,
  sw=512; descale 2^-13 fused into the psum->SBUF bias pass); product =
  xh*Wh + xl*Wh + xh*Wl.  DoubleRow contracts two 128-chunks per pass at
  0.5 cycles/row -> 0.75x the bf16 matmul cost.
  logitsT[t,s] per head: bf16 Q,K, d=64 contraction, two heads packed in
  the PE via tile_position row groups.
  expT = exp(logitsT/8 + mask*NEG_INF) on ACT -- the single-engine
  bottleneck (64 instructions, one Exp table, nothing else runs on ACT).
  ACT paces the whole head loop, so PE filler work (remaining
  projections, V, AV, transposes) is spread one ~0.6us slice per exp
  chunk with explicit deadlines.
  Startup: weight/x DMAs are packed into few fused transfers (each DMA
  issue costs ~650ns of serial SP time) ordered so the first QT matmuls
  start ~4.3us and head 0 runs half-width (s 0-511) exps from ~9.5us;
  startup descales run on ACT (Identity, shares the Exp table) and DVE
  in parallel, with the startup K psum borrowed from the then-idle lg
  pool so it doesn't rotate behind the Q descales.
  AV reoriented: expT chunk is the STATIONARY operand [128t x 128s], V
  (with a memset 1/64 ones column: softmax denominator) is the moving
  operand [128t x 65] -> out[s-part, d|denom] at 65 rows/pass, half the
  cost of the V-stationary orientation; the denominator lands
  per-partition so normalization is one DVE tensor op.
  attn[s,(h d)] -> attnT[(h d),s]: blocks 0-2 via XBAR DMA transpose
  (14ns/tile, no PE/psum; hi/lo fp8 split on the otherwise-idle GPSIMD),
  block 3 on the PE in the tail (XBAR issue latency would gate it).
  Output projection out[s,n] = attnT^T @ Wo in 3-term fp8-DR, split by
  DR blk-pair: pair0 (blks 0-1) runs as late-loop PE filler into bf16
  partials (nh0 pre-scaled for a DVE stt merge, nh1 raw for a PE
  identity-matmul re-inject + ACT copy); pair1 + merges + per-st DMAs
  form the tail. Logits are emitted before each slot's fillers (the
  32-deep in-order PE window would otherwise delay them ~0.8us behind a
  32-matmul av unit and starve ACT).
"""
import sys

sys.path.insert(0, "/opt/trn_rl_repo")

import ml_dtypes
import numpy as np

import concourse.bass as bass
import concourse.mybir as mybir
import concourse.tile as tile
from concourse import bacc
from concourse.bass_utils import run_bass_kernel_spmd

F32 = mybir.dt.float32
BF16 = mybir.dt.bfloat16
E4 = mybir.dt.float8e4
DR = mybir.MatmulPerfMode.DoubleRow
Exp = mybir.ActivationFunctionType.Exp
Ident = mybir.ActivationFunctionType.Identity
MULT = mybir.AluOpType.mult
ADD = mybir.AluOpType.add

B, S, H = 4, 1024, 1024
NH, HD = 16, 64
HPG = 8            # heads per group (per core)
GW = HPG * HD      # 512
NEG_INF = -2.0 ** 32
NCORES = 8
HC = H // 128      # 8 contraction chunks over hidden
TC = S // 128      # 8 chunks over key positions t
SX = 16.0          # x fp8 scale
SW = 512.0         # W fp8 scale
DESCALE = 1.0 / (SX * SW)   # 2^-13


def _build(nrep=1):
    nc = bacc.Bacc("TRN2", target_bir_lowering=False, debug=False)

    # weight layouts are blk-major so per-blk DMA slices are contiguous
    # >=512B runs (sub-512B DMA elements pay 2x latency). x and the qk
    # weights are fused hi|lo / q|k so the startup needs few DMA issues
    # (each costs 650ns of serial SP issue time).
    # x packed as hi|lo-FUSED 256-column pieces (outermost piece dim =>
    # each piece is one contiguous 4KB/partition DMA): all three DR terms
    # of a projection sub-block unlock together as its piece lands, so the
    # PE runs continuously through the startup instead of idling for the
    # lo plane. w0 is chunk-major so one DMA carries wq+wk hi+lo for a
    # chunk range.
    xhl = nc.dram_tensor("xhl", [128, 4, 2, HC, 256], E4,
                         kind="ExternalInput")
    w0 = nc.dram_tensor("w0", [128, HC, 4, 128], E4, kind="ExternalInput")
    wqk = nc.dram_tensor("wqk", [128, 3, 4, HC, 128], E4, kind="ExternalInput")
    wv = nc.dram_tensor("wv", [128, 2, 2, HC, 256], E4, kind="ExternalInput")
    wo = nc.dram_tensor("wo", [128, 2, 4, S], E4, kind="ExternalInput")
    # consts packs [bq | bk | mask] as [128, 4+4+8] (one DMA instead of 3)
    consts = nc.dram_tensor("consts", [128, 16], F32, kind="ExternalInput")
    bv1 = nc.dram_tensor("bv1", [1, GW], F32, kind="ExternalInput")
    ident = nc.dram_tensor("ident", [128, 128], BF16, kind="ExternalInput")
    out = nc.dram_tensor("out", [S, H], BF16, kind="ExternalOutput")

    with tile.TileContext(nc, pool_alloc_mode="stack") as tc:
      for _rep in range(nrep):
        misc_cm = tc.tile_pool(name="misc", bufs=1); misc = misc_cm.__enter__()
        x_cm = tc.tile_pool(name="p_x", bufs=1); p_x = x_cm.__enter__()
        w_cm = tc.tile_pool(name="p_w", bufs=1); p_w = w_cm.__enter__()
        qkt_cm = tc.tile_pool(name="p_qkt", bufs=1); p_qkt = qkt_cm.__enter__()
        v_cm = tc.tile_pool(name="p_v", bufs=1); p_v = v_cm.__enter__()
        exp_cm = tc.tile_pool(name="p_exp", bufs=3); p_exp = exp_cm.__enter__()
        attn_cm = tc.tile_pool(name="p_attn", bufs=1); p_attn = attn_cm.__enter__()
        r4_cm = tc.tile_pool(name="p_r4", bufs=2); p_r4 = r4_cm.__enter__()
        at_cm = tc.tile_pool(name="p_at", bufs=1); p_at = at_cm.__enter__()
        o_cm = tc.tile_pool(name="p_o", bufs=8); p_o = o_cm.__enter__()
        # PSUM: proj(2 banks) + po(2) + lg(4) = 8. lg releases after the
        # head loop; the output-projection pool takes its 4 banks. po stays
        # for the tail transposes (borrowed slots).
        prps_cm = tc.tile_pool(name="ps_proj", bufs=2, space="PSUM")
        ps_proj = prps_cm.__enter__()
        pops_cm = tc.tile_pool(name="ps_po", bufs=2, space="PSUM")
        ps_po = pops_cm.__enter__()
        lgps_cm = tc.tile_pool(name="ps_lg", bufs=2, space="PSUM")
        ps_lg = lgps_cm.__enter__()

        xhl_sb = p_x.tile([128, 4, 2, HC, 256], E4, tag="xhl")
        w0_sb = p_w.tile([128, HC, 4, 128], E4, tag="w0")
        wqk_sb = p_w.tile([128, 3, 4, HC, 128], E4, tag="wqk")
        wv_sb = p_w.tile([128, 2, 2, HC, 256], E4, tag="wv")
        wo_sb = p_w.tile([128, 2, 4, S], E4, tag="wo")
        woh_sb = wo_sb[:, 0]
        wol_sb = wo_sb[:, 1]
        c_sb = misc.tile([128, 16], F32, tag="consts")
        bq_sb = c_sb[:, 0:4]
        bk_sb = c_sb[:, 4:8]
        maskb = misc.tile([128, TC], F32, tag="maskb")
        bv_bc = misc.tile([128, GW], F32, tag="bv")
        id_sb = misc.tile([128, 128], BF16, tag="ident")

        def w_ap(mb, j):
            """Weight blk mb, slot j (0=wqh 1=wql 2=wkh 3=wkl) as
            [128, HC, 128]."""
            if mb == 0:
                return w0_sb[:, :, j, :]
            return wqk_sb[:, mb - 1, j]

        def dma(dst, src):
            nc.sync.dma_start(out=dst, in_=src)

        # Preload the Exp activation table off the critical path: the first
        # real exp would otherwise pay the 1.28us table load at ~10us.
        dummy = misc.tile([128, 1], F32, tag="dummy")
        nc.vector.memset(dummy, 0.0)
        nc.scalar.activation(out=dummy, in_=dummy, func=Exp)

        # Startup DMA queue: the first piece (wqh chunks 0-3) plus the
        # matching xh quarter let the very first QT matmuls start ~4.3us;
        # kernel end time tracks PE-start + PE-busy, so this is critical.
        dma(w0_sb[:, 0:2], w0[:, 0:2])
        dma(xhl_sb[:, 0, :, 0:2, :], xhl[:, 0, :, 0:2, :])
        dma(w0_sb[:, 2:8], w0[:, 2:8])
        dma(xhl_sb[:, 0, :, 2:8, :], xhl[:, 0, :, 2:8, :])
        dma(c_sb, consts[:, :])
        nc.vector.tensor_scalar_mul(maskb, c_sb[:, 8:16], NEG_INF)
        dma(xhl_sb[:, 1], xhl[:, 1])
        dma(xhl_sb[:, 2], xhl[:, 2])
        dma(xhl_sb[:, 3], xhl[:, 3])
        # blk1 weights: QK-mb1 filler halves start at h0c4
        dma(wqk_sb[:, 0], wqk[:, 0])
        # wv first half (heads 0-3): V fillers start at h1c4
        dma(wv_sb[:, 0], wv[:, 0])
        dma(bv_bc, bv1[0:1, :].to_broadcast((128, GW)))
        dma(id_sb, ident[:, :])
        QT = p_qkt.tile([128, 4, S], BF16, tag="QT")
        KT = p_qkt.tile([128, 4, S], BF16, tag="KT")
        V_sb = p_v.tile([128, TC, HPG, HD + 1], BF16, tag="V")
        # softmax-denominator ones column (1/64 scale), built on DVE instead
        # of a 64-descriptor-per-partition DMA (3.6us of DMA queue time)
        nc.vector.memset(V_sb[:, :, :, HD:HD + 1], 1.0 / 64.0)
        dma(wqk_sb[:, 1], wqk[:, 1])
        dma(wqk_sb[:, 2], wqk[:, 2])
        # wv second half (heads 4-7): V-b fillers start at h4c3
        dma(wv_sb[:, 1], wv[:, 1])
        dma(wo_sb, wo[:, :])
        attn_sb = p_attn.tile([128, TC, GW], BF16, tag="attn")
        # attnT: bf16 XBAR-transpose staging + hi/lo fp8 split. Blocks 0-2
        # transpose via DMA (XBAR, 14ns/tile, idle DMA engines mid-loop);
        # block 3 transposes on the PE in the tail (DMA issue latency
        # ~2.9us would gate the tail otherwise).
        attnT_bf = p_at.tile([128, 4, S], BF16, tag="attnTbf")
        attnT_h = p_at.tile([128, 4, S], E4, tag="attnTh")
        attnT_l = p_at.tile([128, 4, S], E4, tag="attnTl")
        # output-projection partial sums (pair0 = attnT blks 0-1, done
        # mid-loop; merged with the tail pair1 psum): [st][nh] -> [128,512]
        part_sb = p_at.tile([128, TC, 2, 512], BF16, tag="part")

        # ---- compute emitters ----
        pq_state = {}

        def proj_qk_half(dst, b_sb, jh, mb, nh, nb, split=False,
                         desc_act=False, use_lg=False):
            """Half (256 s-cols) of a 3-term fp8-DR projection block.
            nb=0 allocates the [128,512] psum tile; nb=1 emits the fused
            descale+bias DVE pass over the full 512. With split=True
            (startup halves) each nb descales its own 256 as soon as its
            group stops -- nb0 on ACT (Copy shares the Exp table), nb1 on
            DVE -- and the xh terms are emitted before the xl terms so
            they flow while the xl half of x is still in flight."""
            wh_t, wl_t = w_ap(mb, jh), w_ap(mb, jh + 1)
            key = (id(dst), mb, nh)
            if split:
                # own psum tile per nb: sharing one tile would serialize
                # nb1's accumulation group behind the engine reading nb0's
                # descale out of the same bank. The K halves of the very
                # first chain borrow the (startup-idle) lg pool so they
                # don't rotate behind the Q descales in the proj pool.
                if use_lg:
                    ps = ps_lg.tile([128, 1024], F32, tag="lg",
                                    name=f"pq{jh}{mb}_{nh}_{nb}")
                    ps = ps[:, 0:512]
                else:
                    ps = ps_proj.tile([128, 512], F32, tag="pproj",
                                      name=f"pq{jh}{mb}_{nh}_{nb}")
            else:
                if nb == 0:
                    pq_state[key] = ps_proj.tile(
                        [128, 512], F32, tag="pproj", name=f"pq{jh}{mb}_{nh}")
                ps = pq_state[key]
            n0 = nh * 512
            pp = nh * 2 + nb   # x piece index (256 s-cols, hi|lo fused)
            os_ = slice(0, 256) if split else slice(nb * 256, (nb + 1) * 256)
            terms = ((wh_t, 0), (wl_t, 0), (wh_t, 1))
            if split:
                order = [(cp, t) for t in terms for cp in range(4)]
            else:
                order = [(cp, t) for cp in range(4) for t in terms]
            for i, (cp, (wt, hl)) in enumerate(order):
                cs = slice(2 * cp, 2 * cp + 2)
                nc.tensor.matmul(
                    ps[:, os_], wt[:, cs, :], xhl_sb[:, pp, hl, cs, :],
                    start=(i == 0), stop=(i == len(order) - 1), perf_mode=DR,
                    skip_group_check=True)
            if split:
                od = dst[:, mb, n0 + nb * 256:n0 + (nb + 1) * 256]
                if nb == 0 and desc_act:
                    # ACT-hosted descale: only before the first exp, while
                    # ACT is idle (later it would queue behind exps)
                    nc.scalar.activation(out=od, in_=ps[:, os_], func=Ident,
                                         bias=b_sb[:, mb:mb + 1],
                                         scale=DESCALE)
                else:
                    nc.vector.tensor_scalar(
                        od, ps[:, os_], DESCALE, b_sb[:, mb:mb + 1],
                        MULT, ADD)
            elif nb == 1:
                nc.vector.tensor_scalar(
                    dst[:, mb, n0:n0 + 512], ps, DESCALE,
                    b_sb[:, mb:mb + 1], MULT, ADD)
                del pq_state[key]

        def proj_v_half(tb, nb):
            """V rows 128*tb for head-group nb (4 heads, 256 wv-cols);
            self-contained: 12 DR matmuls + fused descale+bias pass."""
            ps = ps_proj.tile([128, 512], F32, tag="pproj",
                              name=f"pv{tb}_{nb}")
            pp, to = tb // 2, (tb % 2) * 128
            ts = slice(to, to + 128)
            first = True
            for cp in range(4):
                cs = slice(2 * cp, 2 * cp + 2)
                for wl_, xl_ in ((0, 0), (1, 0), (0, 1)):
                    last = (cp == 3 and xl_ == 1)
                    nc.tensor.matmul(
                        ps[:, 0:256], xhl_sb[:, pp, xl_, cs, ts],
                        wv_sb[:, nb, wl_, cs, :],
                        start=first, stop=last, perf_mode=DR,
                        skip_group_check=True)
                    first = False
            nc.vector.scalar_tensor_tensor(
                V_sb[:, tb, 4 * nb:4 * nb + 4, 0:HD],
                ps[:, 0:256].rearrange("p (h d) -> p h d", h=4), DESCALE,
                bv_bc[:, nb * 256:(nb + 1) * 256].rearrange(
                    "p (h d) -> p h d", h=4), MULT, ADD)

        def logits_exp(h, eT, tcn):
            mb, off = h // 2, (h % 2) * 64
            lg = ps_lg.tile([128, 1024], F32, tag="lg")
            for sh in range(2):
                nc.tensor.matmul(
                    lg[:, sh * 512:(sh + 1) * 512],
                    KT[off:off + 64, mb, tcn * 128:(tcn + 1) * 128],
                    QT[off:off + 64, mb, sh * 512:(sh + 1) * 512],
                    start=True, stop=True, tile_position=(off, 0))
            nc.scalar.activation(
                out=eT[:, tcn, :], in_=lg, func=Exp,
                bias=maskb[:, tcn:tcn + 1], scale=0.125)

        expT = {}
        av_state = {}

        def av_part(h, sbg, tcs):
            """Partial AV accumulation for head h, s-blocks 4*sbg.., over
            the t-chunks in `tcs` (split emission so av(7)'s last chunk is
            the only tail PE work)."""
            key = (h, sbg)
            if key not in av_state:
                av_state[key] = ps_po.tile(
                    [128, 4, HD + 1], F32, tag="po", name=f"po{h}_{sbg}")
            po = av_state[key]
            eT = expT[h]
            for i in range(4):
                sb = sbg * 4 + i
                for tcn in tcs:
                    nc.tensor.matmul(
                        po[:, i, :],
                        eT[:, tcn, sb * 128:(sb + 1) * 128],
                        V_sb[:, tcn, h, :],
                        start=(tcn == 0), stop=(tcn == TC - 1))

        def av_fin(h, sbg):
            po = av_state.pop((h, sbg))
            r4 = p_r4.tile([128, 4, 1], F32, tag="r4")
            nc.vector.reciprocal(r4, po[:, :, HD:HD + 1])
            nc.vector.tensor_mul(
                attn_sb[:, sbg * 4:(sbg + 1) * 4, h * HD:(h + 1) * HD],
                po[:, :, 0:HD], r4[:, :, 0:1].to_broadcast((128, 4, HD)))

        def av(h, sbg):
            av_part(h, sbg, range(TC))
            av_fin(h, sbg)

        def transpose_blk(blk):
            """attn s-block columns of gw-block blk -> attnT hi/lo fp8 rows
            (attn carries a x64 pow2 scale from the 1/64 ones column so the
            values sit in e4m3's normal range; descale folds into the
            output-copy pass). XBAR DMA transpose (no PE, no psum); hi
            split on DVE, lo residual on the otherwise-idle GPSIMD."""
            for sb in range(TC):
                ss = slice(sb * 128, (sb + 1) * 128)
                nc.sync.dma_start_transpose(
                    attnT_bf[:, blk, ss],
                    attn_sb[:, sb, blk * 128:(blk + 1) * 128])
                # both converts on GPSIMD: the ~2.9us XBAR round-trip sits
                # at the head of whichever in-order engine queue hosts the
                # first convert; GPSIMD's queue has nothing time-critical,
                # while a DVE-hosted wait would stall the descale pipeline
                # (psum recycling) behind it
                nc.gpsimd.tensor_copy(attnT_h[:, blk, ss],
                                      attnT_bf[:, blk, ss])
                nc.gpsimd.tensor_sub(attnT_l[:, blk, ss],
                                     attnT_bf[:, blk, ss],
                                     attnT_h[:, blk, ss])

        ODESC = 1.0 / (64.0 * SW)   # attn x64 and wo x512 scales

        def oproj_pair(st, nh, pair, merge_to=None, merge_eng="v", pool=None):
            """6 fp8-DR matmuls of output-projection pair `pair` (attnT
            blk-pair) for s-chunk st, columns nh*512.. .
            pair0 (mid-loop filler): psum -> bf16 partial on DVE. nh0
            stores the partial pre-scaled by ODESC (tail merge is a DVE
            scalar_tensor_tensor); nh1 stores it raw (the tail re-injects
            it into psum via an identity matmul -- GPSIMD can't read PSUM,
            so ACT does that lane's plain scaled copy instead).
            pair1 (tail): per merge_eng "v": DVE stt psum*ODESC+partial;
            "inject": identity-matmul the raw partial into the psum group
            first, then a plain ACT copy*ODESC."""
            sts = slice(st * 128, (st + 1) * 128)
            op = (pool or ps_proj).tile([128, 512], F32, tag="pproj"
                                        if pool is None else "op",
                                        name=f"op{st}_{nh}_{pair}")
            bs = slice(2 * pair, 2 * pair + 2)
            first = True
            if merge_to is not None and merge_eng == "inject":
                nc.tensor.matmul(op, id_sb, part_sb[:, st, nh],
                                 start=True, stop=False,
                                 skip_group_check=True)
                first = False
            for nb in range(2):
                ws = slice(nh * 512 + nb * 256, nh * 512 + (nb + 1) * 256)
                os_ = slice(nb * 256, (nb + 1) * 256)
                for at_t, wo_t in ((attnT_h, woh_sb), (attnT_l, woh_sb),
                                   (attnT_h, wol_sb)):
                    last = (nb == 1 and wo_t is wol_sb)
                    nc.tensor.matmul(
                        op[:, os_], at_t[:, bs, sts], wo_t[:, bs, ws],
                        start=first, stop=last, perf_mode=DR,
                        skip_group_check=True)
                    first = False
            if merge_to is None:
                # partial copy on DVE (NOT Pool: the in-loop proj-psum
                # recycling waits on this copy, Pool's in-order queue is
                # full of non-urgent tp converts, and GPSIMD can't read
                # PSUM on real hardware anyway)
                if nh == 0:
                    nc.vector.tensor_scalar_mul(part_sb[:, st, nh], op,
                                                ODESC)
                else:
                    nc.vector.tensor_copy(part_sb[:, st, nh], op)
            elif merge_eng == "inject":
                nc.scalar.mul(merge_to, op, ODESC)
            else:
                nc.vector.scalar_tensor_tensor(
                    merge_to, op, ODESC, part_sb[:, st, nh], MULT, ADD)

        # ---------------- emission ----------------
        # Startup: QT-nh0 + KT-nb0 (x half0 only) gate half-width exps
        # (s 0-511) for head 0's first four t-chunks, ~4us before a
        # full-width first exp could run (that needs x half1 for QT-nh1).
        # The sh1 half-exps follow once QT-nh1 is projected; tc4-7 run
        # full width. Half-exps pay one extra ACT access-latency each
        # (~0.2us total) for the earlier start.
        def logits_exp_half(h, eT, tcn, sh):
            mb, off = h // 2, (h % 2) * 64
            lg = ps_lg.tile([128, 1024], F32, tag="lg")
            nc.tensor.matmul(
                lg[:, 0:512],
                KT[off:off + 64, mb, tcn * 128:(tcn + 1) * 128],
                QT[off:off + 64, mb, sh * 512:(sh + 1) * 512],
                start=True, stop=True, tile_position=(off, 0))
            nc.scalar.activation(
                out=eT[:, tcn, sh * 512:(sh + 1) * 512], in_=lg[:, 0:512],
                func=Exp, bias=maskb[:, tcn:tcn + 1], scale=0.125)

        expT[0] = p_exp.tile([128, TC, S], BF16, tag="expT", name="expT0")
        proj_qk_half(QT, bq_sb, 0, 0, 0, 0, split=True, desc_act=True)
        proj_qk_half(QT, bq_sb, 0, 0, 0, 1, split=True)
        proj_qk_half(KT, bk_sb, 2, 0, 0, 0, split=True, desc_act=True,
                     use_lg=True)
        logits_exp_half(0, expT[0], 0, 0)
        logits_exp_half(0, expT[0], 1, 0)
        proj_qk_half(KT, bk_sb, 2, 0, 0, 1, split=True, use_lg=True)
        logits_exp_half(0, expT[0], 2, 0)
        logits_exp_half(0, expT[0], 3, 0)
        proj_qk_half(QT, bq_sb, 0, 0, 1, 0)
        proj_qk_half(QT, bq_sb, 0, 0, 1, 1)
        logits_exp_half(0, expT[0], 0, 1)
        logits_exp_half(0, expT[0], 1, 1)
        proj_qk_half(KT, bk_sb, 2, 0, 1, 0)
        logits_exp_half(0, expT[0], 2, 1)
        logits_exp_half(0, expT[0], 3, 1)
        proj_qk_half(KT, bk_sb, 2, 0, 1, 1)

        def F_qk(dst_b, mb, nh, nb):
            dst, b_, jh = (QT, bq_sb, 0) if dst_b == "q" else (KT, bk_sb, 2)
            return lambda: proj_qk_half(dst, b_, jh, mb, nh, nb)

        def F_v(tb, nb):
            return lambda: proj_v_half(tb, nb)

        def F_av(h, g):
            return lambda: av(h, g)

        def F_avp(h, g, tcs):
            return lambda: av_part(h, g, tcs)

        def F_tp(blk):
            return lambda: transpose_blk(blk)

        def F_op0(st, nh):
            return lambda: oproj_pair(st, nh, 0)

        # filler schedule: [h][tcn] -> list of emitters. Budget ~0.6us of
        # PE filler per 1.04us exp chunk; no fillers on the first chunks
        # (their inputs aren't DMA'd yet and PE head-of-line blocking
        # would starve ACT). Deadlines: QT/KT-mb(k) before head 2k's
        # logits, V head-group a (0-3) before av(0) at h2c5, group b
        # before av(4) at h6c4, av(h) done before head h+3 starts (expT
        # pool bufs=3), transpose blk b right after av(2b+1) (XBAR DMA +
        # DVE/GPSIMD only -- no PE cost), output-projection pair0 groups
        # (attnT blks 0-1) as late-loop PE fillers once tp0/tp1 are done,
        # av(7) t-chunks 0-6 in-loop so only its last chunk trails the
        # final exp.
        FILL = {
            (0, 4): [F_qk("q", 1, 0, 0)],
            (0, 5): [F_qk("q", 1, 0, 1)],
            (0, 6): [F_qk("q", 1, 1, 0)],
            (0, 7): [F_qk("q", 1, 1, 1)],
            (1, 0): [F_qk("k", 1, 0, 0)],
            (1, 1): [F_qk("k", 1, 0, 1)],
            (1, 2): [F_qk("k", 1, 1, 0)],
            (1, 3): [F_qk("k", 1, 1, 1)],
            (1, 4): [F_v(0, 0)], (1, 5): [F_v(1, 0)],
            (1, 6): [F_v(2, 0)], (1, 7): [F_v(3, 0)],
            (2, 0): [F_v(4, 0)], (2, 1): [F_v(5, 0)],
            (2, 2): [F_v(6, 0)], (2, 3): [F_v(7, 0)],
            (2, 4): [F_qk("q", 2, 0, 0)],
            (2, 5): [F_av(0, 0)],
            (2, 6): [F_qk("q", 2, 0, 1)],
            (2, 7): [F_av(0, 1)],
            (3, 0): [F_qk("q", 2, 1, 0)],
            (3, 1): [F_av(1, 0)],
            (3, 2): [F_qk("q", 2, 1, 1)],
            (3, 3): [F_qk("k", 2, 0, 0)],
            (3, 4): [F_qk("k", 2, 0, 1)],
            (3, 5): [F_av(1, 1)],
            (3, 6): [F_qk("k", 2, 1, 0), F_tp(0)],
            (3, 7): [F_qk("k", 2, 1, 1)],
            (4, 0): [F_qk("q", 3, 0, 0)],
            (4, 1): [F_av(2, 0)],
            (4, 2): [F_qk("q", 3, 0, 1)],
            (4, 3): [F_v(0, 1)],
            (4, 4): [F_qk("q", 3, 1, 0)],
            (4, 5): [F_av(2, 1)],
            (4, 6): [F_qk("q", 3, 1, 1)],
            (4, 7): [F_v(1, 1)],
            (5, 0): [F_qk("k", 3, 0, 0)],
            (5, 1): [F_av(3, 0)],
            (5, 2): [F_qk("k", 3, 0, 1)],
            (5, 3): [F_qk("k", 3, 1, 0)],
            (5, 4): [F_qk("k", 3, 1, 1)],
            (5, 5): [F_av(3, 1)],
            (5, 6): [F_v(2, 1), F_tp(1)],
            (5, 7): [F_v(3, 1)],
            (6, 0): [F_v(4, 1)],
            (6, 1): [F_v(5, 1)],
            (6, 2): [F_v(6, 1)],
            (6, 3): [F_v(7, 1)],
            (6, 4): [F_av(4, 0)],
            (6, 5): [F_op0(0, 0)],
            (6, 6): [F_av(4, 1)],
            (6, 7): [F_op0(0, 1)],
            (7, 0): [F_av(5, 0)],
            (7, 1): [F_av(5, 1)],
            (7, 2): [F_av(6, 0), F_tp(2)],
            (7, 3): [F_av(6, 1)],
            (7, 4): [F_op0(1, 0), F_op0(1, 1)],
            (7, 5): [F_op0(2, 0), F_op0(2, 1)],
            (7, 6): [F_op0(3, 0), F_op0(3, 1)],
            (7, 7): [F_av(7, 0), F_op0(4, 0), F_op0(4, 1)],
        }

        # logits first within each slot: they enter the PE window ahead of
        # the slot's fillers (the PE drains ready work in dispatch order,
        # so a logits pair emitted after a 32-matmul av unit would wait
        # ~0.86us behind it and starve ACT)
        for h in range(HPG):
            if h > 0:
                expT[h] = p_exp.tile([128, TC, S], BF16, tag="expT",
                                     name=f"expT{h}")
            for tcn in range(TC):
                if h == 0 and tcn < 4:
                    continue  # emitted above as half-width startup exps
                logits_exp(h, expT[h], tcn)
                for f in FILL.get((h, tcn), ()):
                    f()

        lgps_cm.__exit__(None, None, None)
        opps_cm = tc.tile_pool(name="ps_op", bufs=4, space="PSUM")
        ps_op = opps_cm.__enter__()

        def transpose_one3(sb):
            # tail blk3 transposes on the PE (XBAR DMA issue latency
            # ~2.9us/8-issue-serial would gate the tail): psum slots
            # alternate between the AV pool (free after av7) and the proj
            # pool for a 4-wide pipeline; hi copy on ACT (idle after the
            # last exp; Copy shares the Exp table, no reload), lo subtract
            # on DVE.
            if sb % 2:
                bt4 = ps_po.tile([128, 4, HD + 1], F32, tag="po",
                                 name=f"pt3_{sb}")
                pt = bt4[:, 0, :].bitcast(BF16)[:, 0:128]
            else:
                bt = ps_proj.tile([128, 512], F32, tag="pproj",
                                  name=f"pt3_{sb}")
                pt = bt[:, 0:128].bitcast(BF16)[:, 0:128]
            nc.tensor.matmul(
                pt, attn_sb[:, sb, 384:512], id_sb,
                start=True, stop=True, is_transpose=True)
            ss = slice(sb * 128, (sb + 1) * 128)
            nc.scalar.copy(attnT_h[:, 3, ss], pt)
            nc.vector.tensor_sub(attnT_l[:, 3, ss], pt, attnT_h[:, 3, ss])

        # av7 sbg0 was emitted inside slot (7,7) so its tc0-6 matmuls
        # pre-run during the last exps; only sbg1 remains here
        av(7, 1)
        # full-group sts first: their transposes and output DMAs drain
        # while the short pair1+merge sts still compute
        ST_ORDER = [0, 1, 2, 3, 4, 5, 6, 7]
        for sb in ST_ORDER:
            transpose_one3(sb)

        # tail: sts whose pair0 ran as loop filler finish with pair1 +
        # merge (DVE nh0 / GPSIMD nh1); the rest run full 12-matmul
        # groups with plain copies (DVE nh0 / ACT nh1), all in the 4-bank
        # op pool.
        PAIR0_DONE = 5  # sts 0..PAIR0_DONE-1 had pair0 emitted in-loop
        for st in ST_ORDER:
            sts = slice(st * 128, (st + 1) * 128)
            o_sb = p_o.tile([128, 1024], BF16, tag="o")
            if st < PAIR0_DONE:
                oproj_pair(st, 0, 1, merge_to=o_sb[:, 0:512],
                           merge_eng="v", pool=ps_op)
                oproj_pair(st, 1, 1, merge_to=o_sb[:, 512:1024],
                           merge_eng="inject", pool=ps_op)
            else:
                for nh in range(2):
                    # nh1 groups borrow the proj pool (free once the tail
                    # transposes drain) to widen the psum pipeline
                    pool_f = ps_op if nh == 0 else ps_proj
                    op = pool_f.tile([128, 512], F32,
                                     tag="op" if nh == 0 else "pproj",
                                     name=f"opf{st}_{nh}")
                    first = True
                    for nb in range(2):
                        ws = slice(nh * 512 + nb * 256,
                                   nh * 512 + (nb + 1) * 256)
                        os_ = slice(nb * 256, (nb + 1) * 256)
                        for p_ in range(2):
                            for at_t, wo_t in ((attnT_h, woh_sb),
                                               (attnT_l, woh_sb),
                                               (attnT_h, wol_sb)):
                                last = (nb == 1 and p_ == 1
                                        and wo_t is wol_sb)
                                nc.tensor.matmul(
                                    op[:, os_], at_t[:, 2 * p_:2 * p_ + 2, sts],
                                    wo_t[:, 2 * p_:2 * p_ + 2, ws],
                                    start=first, stop=last, perf_mode=DR,
                                    skip_group_check=True)
                                first = False
                    cs2 = slice(nh * 512, (nh + 1) * 512)
                    if nh == 0:
                        nc.vector.tensor_scalar_mul(o_sb[:, cs2], op, ODESC)
                    else:
                        nc.scalar.mul(o_sb[:, cs2], op, ODESC)
            if st != ST_ORDER[-1]:
                dma(out[sts, :], o_sb)
            else:
                # last chunk: per-half DMAs so the final transfer (and the
                # post-DMA fixed latency chain) is half-sized
                dma(out[sts, 0:512], o_sb[:, 0:512])
                dma(out[sts, 512:1024], o_sb[:, 512:1024])

        for cm in (opps_cm, pops_cm, prps_cm, o_cm, at_cm, r4_cm, attn_cm,
                   exp_cm, v_cm, qkt_cm, w_cm, x_cm, misc_cm):
            cm.__exit__(None, None, None)

    nc.compile()
    return nc


_NC = {}


def _get_nc(nrep=1):
    if nrep not in _NC:
        _NC[nrep] = _build(nrep)
    return _NC[nrep]


E4NP = ml_dtypes.float8_e4m3


def _q8(a):
    """fp8 hi/lo split: a ~= hi + lo (both e4m3)."""
    hi = a.astype(E4NP)
    lo = (a - hi.astype(np.float32)).astype(E4NP)
    return hi, lo


def _chunk128(a):
    """[1024, M] -> [128, 8, M] partition-major chunking of the rows."""
    m = a.shape[1]
    return np.ascontiguousarray(a.reshape(HC, 128, m).transpose(1, 0, 2))


def kernel(x, mask, Wq, bq, Wk, bk, Wv, bv, Wo, bo, _trace=False):
    x = np.asarray(x, dtype=np.float32)
    mask = np.asarray(mask, dtype=np.float32)
    Wq, Wk, Wv, Wo = (np.asarray(w, dtype=np.float32) for w in (Wq, Wk, Wv, Wo))
    bq, bk, bv, bo = (np.asarray(b_, dtype=np.float32) for b_ in (bq, bk, bv, bo))

    nc = _get_nc()
    # The on-device ones column is memset to 1/64: the softmax denominator
    # comes out pre-scaled so the normalized attn carries a x64 factor,
    # putting it in e4m3's normal range for the fp8 output projection
    # (descaled by ODESC at the end).
    ident = np.eye(128, dtype=ml_dtypes.bfloat16)
    in_maps = []
    for c in range(NCORES):
        b, g = c // 2, c % 2
        sl = slice(g * GW, (g + 1) * GW)
        xh_, xl_ = _q8(np.ascontiguousarray(x[b].T) * SX)
        wq_h, wq_l = _q8(Wq[:, sl] * SW)
        wk_h, wk_l = _q8(Wk[:, sl] * SW)
        wv_h, wv_l = _q8(Wv[:, sl] * SW)
        wo_h, wo_l = _q8(np.ascontiguousarray(
            Wo[sl, :].reshape(4, 128, S).transpose(1, 0, 2)) * SW)

        def wblk(a, nblk):
            # [1024, 512] -> [128, nblk, HC, 512//nblk] (partition-major
            # rows, blk-major cols so per-blk DMA slices are contiguous)
            cc = a.reshape(HC, 128, nblk, GW // nblk)
            return np.ascontiguousarray(cc.transpose(1, 2, 0, 3))

        qh, ql = wblk(wq_h, 4), wblk(wq_l, 4)
        kh, kl = wblk(wk_h, 4), wblk(wk_l, 4)
        vh, vl = wblk(wv_h, 2), wblk(wv_l, 2)
        # w0: blk0 of [wqh|wql|wkh|wkl] chunk-major; wqk: blks 1-3 stacked
        w0 = np.stack([a[:, 0] for a in (qh, ql, kh, kl)], axis=2)
        wqk = np.stack(
            [np.stack([a[:, blk] for a in (qh, ql, kh, kl)], axis=1)
             for blk in range(1, 4)], axis=1)
        wv_f = np.stack(
            [np.stack([vh[:, nb], vl[:, nb]], axis=1) for nb in range(2)],
            axis=1)
        # x as 4 hi|lo-fused 256-column pieces: [128, 4, 2, HC, 256]
        xhl = np.stack(
            [_chunk128(xh_).reshape(128, HC, 4, 256).transpose(0, 2, 1, 3),
             _chunk128(xl_).reshape(128, HC, 4, 256).transpose(0, 2, 1, 3)],
            axis=2)

        in_maps.append({
            "xhl": np.ascontiguousarray(xhl),
            "w0": np.ascontiguousarray(w0),
            "wqk": np.ascontiguousarray(wqk),
            "wv": np.ascontiguousarray(wv_f),
            "wo": np.ascontiguousarray(np.stack([wo_h, wo_l], axis=1)),
            "consts": np.ascontiguousarray(np.concatenate([
                bq[sl].reshape(4, 128).T, bk[sl].reshape(4, 128).T,
                mask[b, 0, 0, :].reshape(8, 128).T], axis=1)),
            "bv1": np.ascontiguousarray(bv[sl]).reshape(1, GW),
            "ident": ident,
        })
    # First execution after NEFF load can race engine table initialization.
    # Warm up, then run.
    run_bass_kernel_spmd(nc, in_maps, core_ids=list(range(NCORES)))
    res = run_bass_kernel_spmd(
        nc, in_maps, core_ids=list(range(NCORES)), trace=_trace)
    kernel.last_results = res
    parts = [res.results[c]["out"].astype(np.float32) for c in range(NCORES)]
    return np.stack(
        [parts[2 * b] + parts[2 * b + 1] + bo for b in range(B)]
    ).astype(np.float32)
